# revision 6
# baseline (speedup 1.0000x reference)
"""LoRA Linear kernel for Trainium2, 8 NeuronCores.

Computes out = x @ (W + lora_A @ lora_B)^T + bias for
x [4, 2048, 4096], W [4096, 4096], lora_A [4096, 16], lora_B [16, 4096].

Sharding: 2-way over tokens (M = 8192 -> 4096/core) x 4-way over
out_features (4096 -> 1024/core). Host pre-transposes/pre-tiles x and W
so every DMA is a large contiguous 2D transfer with the contraction dim
(in_features) on partitions; the LoRA delta is folded into W^T on-device
with rank-16 matmuls (Wtot^T = W^T + B^T A^T), interleaved with the
first three token tiles' accumulation. Operands are bf16 (fp32 PSUM
accumulation), outputs stored bf16 and upcast on host.

Schedule notes (v2): the W^T stream (8 MiB/core) is the critical
startup dependency - it is front-loaded on the vector+scalar queues
while x tiles ride sync+gpsimd, the first x tile is split into 4
sub-DMAs so the PE starts at ~4us, and main tiles run os-outer so the
first output half stores while the second accumulates (halves the
tail).
"""

import ml_dtypes

import numpy as np

import concourse.bass as bass
import concourse.bacc as bacc
import concourse.mybir as mybir
import concourse.tile as tile
from concourse.bass_utils import run_bass_kernel_spmd

IN_F = 4096
OUT_F = 4096
RANK = 16
BATCH, SEQ = 4, 2048
M_TOT = BATCH * SEQ          # 8192 tokens
MG, OG = 2, 4                # shard grid: token-groups x outfeature-groups
M_LOC = M_TOT // MG          # 4096 tokens per core
O_LOC = OUT_F // OG          # 1024 out features per core
P = 128
KI = IN_F // P               # 32 contraction tiles
NF = 512                     # matmul moving free dim (one PSUM bank)
OS = O_LOC // NF             # 2 output column passes
MT = M_LOC // P              # 32 token tiles per core

F32 = mybir.dt.float32
BF16 = mybir.dt.bfloat16

_cache = {}


def _build():
    nc = bacc.Bacc(None, target_bir_lowering=False)

    # x pre-tiled on host to [MT, P, KI, P]: (mt, i_within, i_tile, m)
    xt = nc.dram_tensor("xt", [MT, P, KI, P], BF16, kind="ExternalInput")
    wt = nc.dram_tensor("wt", [IN_F, O_LOC], BF16, kind="ExternalInput")
    lb = nc.dram_tensor("lb", [RANK, IN_F], F32, kind="ExternalInput")
    at = nc.dram_tensor("at", [RANK, O_LOC], F32, kind="ExternalInput")
    br = nc.dram_tensor("br", [P, O_LOC], F32, kind="ExternalInput")
    out = nc.dram_tensor("out", [M_LOC, O_LOC], BF16, kind="ExternalOutput")

    with tile.TileContext(nc) as tc:
        with (
            tc.tile_pool(name="const", bufs=1) as const_pool,
            tc.tile_pool(name="xin", bufs=4) as xin_pool,
            tc.tile_pool(name="outs", bufs=4) as out_pool,
            tc.tile_pool(name="psum", bufs=2, space="PSUM") as psum_pool,
            tc.tile_pool(name="psum_mm", bufs=3, space="PSUM") as psum_mm_pool,
        ):
            # resident folded weight, [i_within, i_tile, o] = W^T + B^T A^T
            wtot = const_pool.tile([P, KI, O_LOC], BF16, name="wtot")
            a_raw = const_pool.tile([RANK, O_LOC], F32, name="a_raw")
            a_sb = const_pool.tile([RANK, O_LOC], BF16, name="a_sb")
            b_raw = const_pool.tile([RANK, IN_F], F32, name="b_raw")
            b_sb = const_pool.tile([RANK, IN_F], BF16, name="b_sb")
            bias_sb = const_pool.tile([P, O_LOC], F32, name="bias_sb")

            # ---- DMA issue order matters: per-queue FIFO (only sync /
            # scalar / gpsimd can issue DMAs). The 8 MiB W^T stream is the
            # critical startup dependency: stripe it over all 3 queues,
            # ahead of the steady-state x tiles.
            nc.gpsimd.dma_start(a_raw[:], at[:])
            nc.gpsimd.dma_start(b_raw[:], lb[:])
            nc.vector.tensor_copy(out=a_sb[:], in_=a_raw[:])
            nc.vector.tensor_copy(out=b_sb[:], in_=b_raw[:])

            x_tiles = {}

            def load_x(mt):
                x_tile = xin_pool.tile([P, KI, P], BF16, name="x_tile", tag="x_tile")
                if mt == 0:
                    for q in range(4):
                        nc.sync.dma_start(
                            x_tile[:, q * 8 : (q + 1) * 8, :],
                            xt[0, :, q * 8 : (q + 1) * 8, :],
                        )
                else:
                    eng = nc.sync if mt % 2 == 0 else nc.gpsimd
                    eng.dma_start(x_tile[:], xt[mt])
                return x_tile

            # W^T chunks land directly in wtot; the fold then adds the
            # rank-16 delta in place.
            x_tiles[0] = load_x(0)
            x_tiles[1] = load_x(1)
            x_tiles[2] = load_x(2)
            w_engines = [nc.scalar, nc.gpsimd, nc.sync]
            for ki in range(KI):
                w_engines[ki % 3].dma_start(
                    wtot[:, ki, :], wt[ki * P : (ki + 1) * P, :]
                )
            nc.gpsimd.dma_start(bias_sb[:], br[:])

            def mm_pair(x_tile, ki, psums):
                for os_ in range(OS):
                    nc.tensor.matmul(
                        psums[os_][:],
                        x_tile[:, ki, :],
                        wtot[:, ki, os_ * NF : (os_ + 1) * NF],
                        start=(ki == 0),
                        stop=(ki == KI - 1),
                    )

            def store_one(mt, os_, psum):
                o_tile = out_pool.tile([P, NF], BF16, name="o_tile", tag="o_tile")
                nc.vector.tensor_add(
                    out=o_tile[:],
                    in0=psum[:],
                    in1=bias_sb[:, os_ * NF : (os_ + 1) * NF],
                )
                nc.scalar.dma_start(
                    out[mt * P : (mt + 1) * P, os_ * NF : (os_ + 1) * NF],
                    o_tile[:],
                )

            # ---- m_tiles 0..2, interleaved with the W fold ----
            NLEAD = 3
            lead_psums = [
                [
                    psum_mm_pool.tile(
                        [P, NF], F32, name=f"psum_{mt}_{os_}", tag=f"ps{os_}"
                    )
                    for os_ in range(OS)
                ]
                for mt in range(NLEAD)
            ]
            for ki in range(KI):
                b_ap = b_sb[:, ki * P : (ki + 1) * P]
                for os_ in range(OS):
                    dpsum = psum_pool.tile([P, NF], F32, name="dpsum", tag="dpsum")
                    nc.tensor.matmul(
                        dpsum[:],
                        b_ap,
                        a_sb[:, os_ * NF : (os_ + 1) * NF],
                        start=True,
                        stop=True,
                    )
                    nc.vector.tensor_add(
                        out=wtot[:, ki, os_ * NF : (os_ + 1) * NF],
                        in0=dpsum[:],
                        in1=wtot[:, ki, os_ * NF : (os_ + 1) * NF],
                    )
                for mt in range(NLEAD):
                    mm_pair(x_tiles[mt], ki, lead_psums[mt])
            for mt in range(NLEAD):
                for os_ in range(OS):
                    store_one(mt, os_, lead_psums[mt][os_])

            # ---- m_tiles NLEAD..MT-1, os-outer so os0 stores overlap os1 ----
            for mt in range(NLEAD, MT):
                x_tile = load_x(mt)
                for os_ in range(OS):
                    psum = psum_mm_pool.tile(
                        [P, NF], F32, name=f"psum{os_}", tag=f"ps{os_}"
                    )
                    for ki in range(KI):
                        nc.tensor.matmul(
                            psum[:],
                            x_tile[:, ki, :],
                            wtot[:, ki, os_ * NF : (os_ + 1) * NF],
                            start=(ki == 0),
                            stop=(ki == KI - 1),
                        )
                    store_one(mt, os_, psum)
    nc.finalize()
    return nc


def kernel(x, W, bias, lora_A, lora_B):
    x = np.asarray(x, dtype=np.float32)
    W = np.asarray(W, dtype=np.float32)
    bias = np.asarray(bias, dtype=np.float32)
    lora_A = np.asarray(lora_A, dtype=np.float32)
    lora_B = np.asarray(lora_B, dtype=np.float32)

    if "nc" not in _cache:
        _cache["nc"] = _build()
    nc = _cache["nc"]

    xr = x.reshape(M_TOT, IN_F).astype(ml_dtypes.bfloat16)
    in_maps = []
    for c in range(8):
        mg, og = c % MG, c // MG
        xs = xr[mg * M_LOC : (mg + 1) * M_LOC]
        # [M_LOC, IN_F] -> (mt, m, ki, p) -> (mt, p, ki, m)
        xs = np.ascontiguousarray(xs.reshape(MT, P, KI, P).transpose(0, 3, 2, 1))
        in_maps.append(
            {
                "xt": xs,
                "wt": np.ascontiguousarray(W[og * O_LOC : (og + 1) * O_LOC].T.astype(ml_dtypes.bfloat16)),
                "lb": np.ascontiguousarray(lora_B),
                "at": np.ascontiguousarray(lora_A[og * O_LOC : (og + 1) * O_LOC].T),
                "br": np.ascontiguousarray(
                    np.broadcast_to(bias[og * O_LOC : (og + 1) * O_LOC], (P, O_LOC))
                ),
            }
        )

    res = run_bass_kernel_spmd(nc, in_maps, core_ids=list(range(8)))

    out = np.empty((M_TOT, OUT_F), dtype=np.float32)
    for c in range(8):
        mg, og = c % MG, c // MG
        out[mg * M_LOC : (mg + 1) * M_LOC, og * O_LOC : (og + 1) * O_LOC] = res.results[
            c
        ]["out"].astype(np.float32)
    return out.reshape(BATCH, SEQ, OUT_F)


# revision 7
# speedup vs baseline: 1.0120x; 1.0120x over previous
"""LoRA Linear kernel for Trainium2, 8 NeuronCores.

Computes out = x @ (W + lora_A @ lora_B)^T + bias for
x [4, 2048, 4096], W [4096, 4096], lora_A [4096, 16], lora_B [16, 4096].

Sharding: 2-way over tokens (M = 8192 -> 4096/core) x 4-way over
out_features (4096 -> 1024/core). Host pre-transposes/pre-tiles x and W
so every DMA is a large contiguous 2D transfer with the contraction dim
(in_features) on partitions; the LoRA delta is folded into W^T on-device
with rank-16 matmuls (Wtot^T = W^T + B^T A^T), interleaved with the
first three token tiles' accumulation. Operands are bf16 (fp32 PSUM
accumulation), outputs stored bf16 and upcast on host.

Schedule notes (v2): the W^T stream (8 MiB/core) is the critical
startup dependency - it is front-loaded on the vector+scalar queues
while x tiles ride sync+gpsimd, the first x tile is split into 4
sub-DMAs so the PE starts at ~4us, and main tiles run os-outer so the
first output half stores while the second accumulates (halves the
tail).
"""

import ml_dtypes

import numpy as np

import concourse.bass as bass
import concourse.bacc as bacc
import concourse.mybir as mybir
import concourse.tile as tile
from concourse.bass_utils import run_bass_kernel_spmd

IN_F = 4096
OUT_F = 4096
RANK = 16
BATCH, SEQ = 4, 2048
M_TOT = BATCH * SEQ          # 8192 tokens
MG, OG = 2, 4                # shard grid: token-groups x outfeature-groups
M_LOC = M_TOT // MG          # 4096 tokens per core
O_LOC = OUT_F // OG          # 1024 out features per core
P = 128
KI = IN_F // P               # 32 contraction tiles
NF = 512                     # matmul moving free dim (one PSUM bank)
OS = O_LOC // NF             # 2 output column passes
MT = M_LOC // P              # 32 token tiles per core

F32 = mybir.dt.float32
BF16 = mybir.dt.bfloat16

_cache = {}


def _build():
    nc = bacc.Bacc(None, target_bir_lowering=False)

    # x pre-tiled on host to [MT, P, KI, P]: (mt, i_within, i_tile, m)
    xt = nc.dram_tensor("xt", [MT, P, KI, P], BF16, kind="ExternalInput")
    wt = nc.dram_tensor("wt", [IN_F, O_LOC], BF16, kind="ExternalInput")
    lb = nc.dram_tensor("lb", [RANK, IN_F], F32, kind="ExternalInput")
    at = nc.dram_tensor("at", [RANK, O_LOC], F32, kind="ExternalInput")
    br = nc.dram_tensor("br", [P, O_LOC], F32, kind="ExternalInput")
    out = nc.dram_tensor("out", [M_LOC, O_LOC], BF16, kind="ExternalOutput")

    with tile.TileContext(nc) as tc:
        with (
            tc.tile_pool(name="const", bufs=1) as const_pool,
            tc.tile_pool(name="xin", bufs=4) as xin_pool,
            tc.tile_pool(name="outs", bufs=4) as out_pool,
            tc.tile_pool(name="psum", bufs=2, space="PSUM") as psum_pool,
            tc.tile_pool(name="psum_mm", bufs=3, space="PSUM") as psum_mm_pool,
        ):
            # resident folded weight, [i_within, i_tile, o] = W^T + B^T A^T
            wtot = const_pool.tile([P, KI, O_LOC], BF16, name="wtot")
            a_raw = const_pool.tile([RANK, O_LOC], F32, name="a_raw")
            a_sb = const_pool.tile([RANK, O_LOC], BF16, name="a_sb")
            b_raw = const_pool.tile([RANK, IN_F], F32, name="b_raw")
            b_sb = const_pool.tile([RANK, IN_F], BF16, name="b_sb")
            bias_sb = const_pool.tile([P, O_LOC], F32, name="bias_sb")

            # ---- DMA issue order matters: per-queue FIFO (only sync /
            # scalar / gpsimd can issue DMAs). Lead x tiles must land
            # first (the Tensor queue head-of-line blocks on them), then
            # the 8 MiB W^T stream: the first 11 ki chunks ride the
            # otherwise-idle scalar queue in order, the rest stripe over
            # gpsimd/sync behind the lead x tiles.
            x_tiles = {}

            def load_x(mt):
                x_tile = xin_pool.tile([P, KI, P], BF16, name="x_tile", tag="x_tile")
                eng = nc.sync if mt % 2 == 0 else nc.gpsimd
                eng.dma_start(x_tile[:], xt[mt])
                return x_tile

            nc.gpsimd.dma_start(a_raw[:], at[:])
            nc.gpsimd.dma_start(b_raw[:], lb[:])
            nc.vector.tensor_copy(out=a_sb[:], in_=a_raw[:])
            nc.vector.tensor_copy(out=b_sb[:], in_=b_raw[:])
            x_tiles[0] = load_x(0)
            x_tiles[1] = load_x(1)
            x_tiles[2] = load_x(2)

            # W^T chunks land directly in wtot; the fold then adds the
            # rank-16 delta in place.
            NW_SCALAR = 11
            for ki in range(NW_SCALAR):
                nc.scalar.dma_start(wtot[:, ki, :], wt[ki * P : (ki + 1) * P, :])
            for j, ki in enumerate(range(NW_SCALAR, KI)):
                eng = nc.gpsimd if j % 2 == 0 else nc.sync
                eng.dma_start(wtot[:, ki, :], wt[ki * P : (ki + 1) * P, :])
            nc.gpsimd.dma_start(bias_sb[:], br[:])

            def mm_pair(x_tile, ki, psums):
                for os_ in range(OS):
                    nc.tensor.matmul(
                        psums[os_][:],
                        x_tile[:, ki, :],
                        wtot[:, ki, os_ * NF : (os_ + 1) * NF],
                        start=(ki == 0),
                        stop=(ki == KI - 1),
                    )

            def store_one(mt, os_, psum):
                o_tile = out_pool.tile([P, NF], BF16, name="o_tile", tag="o_tile")
                nc.vector.tensor_add(
                    out=o_tile[:],
                    in0=psum[:],
                    in1=bias_sb[:, os_ * NF : (os_ + 1) * NF],
                )
                nc.scalar.dma_start(
                    out[mt * P : (mt + 1) * P, os_ * NF : (os_ + 1) * NF],
                    o_tile[:],
                )

            # ---- m_tiles 0..2, interleaved with the W fold ----
            NLEAD = 3
            lead_psums = [
                [
                    psum_mm_pool.tile(
                        [P, NF], F32, name=f"psum_{mt}_{os_}", tag=f"ps{os_}"
                    )
                    for os_ in range(OS)
                ]
                for mt in range(NLEAD)
            ]
            for ki in range(KI):
                b_ap = b_sb[:, ki * P : (ki + 1) * P]
                for os_ in range(OS):
                    dpsum = psum_pool.tile([P, NF], F32, name="dpsum", tag="dpsum")
                    nc.tensor.matmul(
                        dpsum[:],
                        b_ap,
                        a_sb[:, os_ * NF : (os_ + 1) * NF],
                        start=True,
                        stop=True,
                    )
                    nc.vector.tensor_add(
                        out=wtot[:, ki, os_ * NF : (os_ + 1) * NF],
                        in0=dpsum[:],
                        in1=wtot[:, ki, os_ * NF : (os_ + 1) * NF],
                    )
                for mt in range(NLEAD):
                    mm_pair(x_tiles[mt], ki, lead_psums[mt])
            for mt in range(NLEAD):
                for os_ in range(OS):
                    store_one(mt, os_, lead_psums[mt][os_])

            # ---- m_tiles NLEAD..MT-1, os-outer so os0 stores overlap os1 ----
            for mt in range(NLEAD, MT):
                x_tile = load_x(mt)
                for os_ in range(OS):
                    psum = psum_mm_pool.tile(
                        [P, NF], F32, name=f"psum{os_}", tag=f"ps{os_}"
                    )
                    for ki in range(KI):
                        nc.tensor.matmul(
                            psum[:],
                            x_tile[:, ki, :],
                            wtot[:, ki, os_ * NF : (os_ + 1) * NF],
                            start=(ki == 0),
                            stop=(ki == KI - 1),
                        )
                    store_one(mt, os_, psum)
    nc.finalize()
    return nc


def kernel(x, W, bias, lora_A, lora_B):
    x = np.asarray(x, dtype=np.float32)
    W = np.asarray(W, dtype=np.float32)
    bias = np.asarray(bias, dtype=np.float32)
    lora_A = np.asarray(lora_A, dtype=np.float32)
    lora_B = np.asarray(lora_B, dtype=np.float32)

    if "nc" not in _cache:
        _cache["nc"] = _build()
    nc = _cache["nc"]

    xr = x.reshape(M_TOT, IN_F).astype(ml_dtypes.bfloat16)
    in_maps = []
    for c in range(8):
        mg, og = c % MG, c // MG
        xs = xr[mg * M_LOC : (mg + 1) * M_LOC]
        # [M_LOC, IN_F] -> (mt, m, ki, p) -> (mt, p, ki, m)
        xs = np.ascontiguousarray(xs.reshape(MT, P, KI, P).transpose(0, 3, 2, 1))
        in_maps.append(
            {
                "xt": xs,
                "wt": np.ascontiguousarray(W[og * O_LOC : (og + 1) * O_LOC].T.astype(ml_dtypes.bfloat16)),
                "lb": np.ascontiguousarray(lora_B),
                "at": np.ascontiguousarray(lora_A[og * O_LOC : (og + 1) * O_LOC].T),
                "br": np.ascontiguousarray(
                    np.broadcast_to(bias[og * O_LOC : (og + 1) * O_LOC], (P, O_LOC))
                ),
            }
        )

    res = run_bass_kernel_spmd(nc, in_maps, core_ids=list(range(8)))

    out = np.empty((M_TOT, OUT_F), dtype=np.float32)
    for c in range(8):
        mg, og = c % MG, c // MG
        out[mg * M_LOC : (mg + 1) * M_LOC, og * O_LOC : (og + 1) * O_LOC] = res.results[
            c
        ]["out"].astype(np.float32)
    return out.reshape(BATCH, SEQ, OUT_F)


# revision 8
# speedup vs baseline: 1.0127x; 1.0007x over previous
"""LoRA Linear kernel for Trainium2, 8 NeuronCores.

Computes out = x @ (W + lora_A @ lora_B)^T + bias for
x [4, 2048, 4096], W [4096, 4096], lora_A [4096, 16], lora_B [16, 4096].

Sharding: 2-way over tokens (M = 8192 -> 4096/core) x 4-way over
out_features (4096 -> 1024/core). Host pre-transposes/pre-tiles x and W
so every DMA is a large contiguous 2D transfer with the contraction dim
(in_features) on partitions; the LoRA delta is folded into W^T on-device
with rank-16 matmuls (Wtot^T = W^T + B^T A^T). Operands are bf16 (fp32
PSUM accumulation), outputs stored bf16 and upcast on host.

Schedule (v4): the startup critical path is the 8 MiB W^T stream - it
gets DMA-queue priority right behind the first two x tiles, striped
over all three DMA-capable queues, landing directly in wtot (the fold
adds the delta in place). Two lead token-tiles interleave with the fold
at ki pace to cover the stream latency; the remaining 30 tiles run
os-outer (first output half stores while the second accumulates) from a
6-deep PSUM rotation so accumulation-group starts never wait on bank
recycling.
"""

import ml_dtypes

import numpy as np

import concourse.bass as bass
import concourse.bacc as bacc
import concourse.mybir as mybir
import concourse.tile as tile
from concourse.bass_utils import run_bass_kernel_spmd

IN_F = 4096
OUT_F = 4096
RANK = 16
BATCH, SEQ = 4, 2048
M_TOT = BATCH * SEQ          # 8192 tokens
MG, OG = 2, 4                # shard grid: token-groups x outfeature-groups
M_LOC = M_TOT // MG          # 4096 tokens per core
O_LOC = OUT_F // OG          # 1024 out features per core
P = 128
KI = IN_F // P               # 32 contraction tiles
NF = 512                     # matmul moving free dim (one PSUM bank)
OS = O_LOC // NF             # 2 output column passes
MT = M_LOC // P              # 32 token tiles per core
NLEAD = 2

F32 = mybir.dt.float32
BF16 = mybir.dt.bfloat16

_cache = {}


def _build():
    nc = bacc.Bacc(None, target_bir_lowering=False)

    # x pre-tiled on host to [MT, P, KI, P]: (mt, i_within, i_tile, m)
    xt = nc.dram_tensor("xt", [MT, P, KI, P], BF16, kind="ExternalInput")
    wt = nc.dram_tensor("wt", [IN_F, O_LOC], BF16, kind="ExternalInput")
    lb = nc.dram_tensor("lb", [RANK, IN_F], F32, kind="ExternalInput")
    at = nc.dram_tensor("at", [RANK, O_LOC], F32, kind="ExternalInput")
    br = nc.dram_tensor("br", [P, O_LOC], F32, kind="ExternalInput")
    out = nc.dram_tensor("out", [M_LOC, O_LOC], BF16, kind="ExternalOutput")

    with tile.TileContext(nc) as tc:
        with (
            tc.tile_pool(name="const", bufs=1) as const_pool,
            tc.tile_pool(name="xin", bufs=4) as xin_pool,
            tc.tile_pool(name="outs", bufs=4) as out_pool,
            tc.tile_pool(name="psum", bufs=2, space="PSUM") as psum_pool,
            tc.tile_pool(name="psum_mm", bufs=6, space="PSUM") as psum_mm_pool,
        ):
            # resident folded weight, [i_within, i_tile, o] = W^T + B^T A^T
            wtot = const_pool.tile([P, KI, O_LOC], BF16, name="wtot")
            a_raw = const_pool.tile([RANK, O_LOC], F32, name="a_raw")
            a_sb = const_pool.tile([RANK, O_LOC], BF16, name="a_sb")
            b_raw = const_pool.tile([RANK, IN_F], F32, name="b_raw")
            b_sb = const_pool.tile([RANK, IN_F], BF16, name="b_sb")
            bias_sb = const_pool.tile([P, O_LOC], F32, name="bias_sb")

            x_tiles = {}

            def load_x(mt):
                x_tile = xin_pool.tile([P, KI, P], BF16, name="x_tile", tag="x_tile")
                eng = nc.sync if mt % 2 == 0 else nc.gpsimd
                eng.dma_start(x_tile[:], xt[mt])
                return x_tile

            # ---- DMA issue order matters: per-queue FIFO (only sync /
            # scalar / gpsimd can issue DMAs). Priority: fold constants,
            # the two lead x tiles, then the whole W^T stream striped over
            # all three queues, then steady-state x.
            nc.gpsimd.dma_start(a_raw[:], at[:])
            nc.gpsimd.dma_start(b_raw[:], lb[:])
            nc.vector.tensor_copy(out=a_sb[:], in_=a_raw[:])
            nc.vector.tensor_copy(out=b_sb[:], in_=b_raw[:])
            x_tiles[0] = load_x(0)
            x_tiles[1] = load_x(1)

            # W^T chunks land directly in wtot; the fold adds the rank-16
            # delta in place.
            w_engines = [nc.scalar, nc.sync, nc.gpsimd]
            for ki in range(KI):
                w_engines[ki % 3].dma_start(
                    wtot[:, ki, :], wt[ki * P : (ki + 1) * P, :]
                )
            nc.gpsimd.dma_start(bias_sb[:], br[:])

            def store_one(mt, os_, psum):
                o_tile = out_pool.tile([P, NF], BF16, name="o_tile", tag="o_tile")
                nc.vector.tensor_add(
                    out=o_tile[:],
                    in0=psum[:],
                    in1=bias_sb[:, os_ * NF : (os_ + 1) * NF],
                )
                nc.scalar.dma_start(
                    out[mt * P : (mt + 1) * P, os_ * NF : (os_ + 1) * NF],
                    o_tile[:],
                )

            # ---- m_tiles 0..NLEAD-1, interleaved with the W fold ----
            lead_psums = [
                [
                    psum_mm_pool.tile([P, NF], F32, name=f"psum_{mt}_{os_}", tag="mm")
                    for os_ in range(OS)
                ]
                for mt in range(NLEAD)
            ]
            for ki in range(KI):
                b_ap = b_sb[:, ki * P : (ki + 1) * P]
                for os_ in range(OS):
                    dpsum = psum_pool.tile([P, NF], F32, name="dpsum", tag="dpsum")
                    nc.tensor.matmul(
                        dpsum[:],
                        b_ap,
                        a_sb[:, os_ * NF : (os_ + 1) * NF],
                        start=True,
                        stop=True,
                    )
                    nc.vector.tensor_add(
                        out=wtot[:, ki, os_ * NF : (os_ + 1) * NF],
                        in0=dpsum[:],
                        in1=wtot[:, ki, os_ * NF : (os_ + 1) * NF],
                    )
                for mt in range(NLEAD):
                    for os_ in range(OS):
                        nc.tensor.matmul(
                            lead_psums[mt][os_][:],
                            x_tiles[mt][:, ki, :],
                            wtot[:, ki, os_ * NF : (os_ + 1) * NF],
                            start=(ki == 0),
                            stop=(ki == KI - 1),
                        )
            for mt in range(NLEAD):
                for os_ in range(OS):
                    store_one(mt, os_, lead_psums[mt][os_])

            # ---- m_tiles NLEAD..MT-1, os-outer so os0 stores overlap os1 ----
            for mt in range(NLEAD, MT):
                x_tile = load_x(mt)
                for os_ in range(OS):
                    psum = psum_mm_pool.tile([P, NF], F32, name=f"psum{os_}", tag="mm")
                    for ki in range(KI):
                        nc.tensor.matmul(
                            psum[:],
                            x_tile[:, ki, :],
                            wtot[:, ki, os_ * NF : (os_ + 1) * NF],
                            start=(ki == 0),
                            stop=(ki == KI - 1),
                        )
                    store_one(mt, os_, psum)
    nc.finalize()
    return nc


def kernel(x, W, bias, lora_A, lora_B):
    x = np.asarray(x, dtype=np.float32)
    W = np.asarray(W, dtype=np.float32)
    bias = np.asarray(bias, dtype=np.float32)
    lora_A = np.asarray(lora_A, dtype=np.float32)
    lora_B = np.asarray(lora_B, dtype=np.float32)

    if "nc" not in _cache:
        _cache["nc"] = _build()
    nc = _cache["nc"]

    xr = x.reshape(M_TOT, IN_F).astype(ml_dtypes.bfloat16)
    in_maps = []
    for c in range(8):
        mg, og = c % MG, c // MG
        xs = xr[mg * M_LOC : (mg + 1) * M_LOC]
        # [M_LOC, IN_F] -> (mt, m, ki, p) -> (mt, p, ki, m)
        xs = np.ascontiguousarray(xs.reshape(MT, P, KI, P).transpose(0, 3, 2, 1))
        in_maps.append(
            {
                "xt": xs,
                "wt": np.ascontiguousarray(W[og * O_LOC : (og + 1) * O_LOC].T.astype(ml_dtypes.bfloat16)),
                "lb": np.ascontiguousarray(lora_B),
                "at": np.ascontiguousarray(lora_A[og * O_LOC : (og + 1) * O_LOC].T),
                "br": np.ascontiguousarray(
                    np.broadcast_to(bias[og * O_LOC : (og + 1) * O_LOC], (P, O_LOC))
                ),
            }
        )

    res = run_bass_kernel_spmd(nc, in_maps, core_ids=list(range(8)))

    out = np.empty((M_TOT, OUT_F), dtype=np.float32)
    for c in range(8):
        mg, og = c % MG, c // MG
        out[mg * M_LOC : (mg + 1) * M_LOC, og * O_LOC : (og + 1) * O_LOC] = res.results[
            c
        ]["out"].astype(np.float32)
    return out.reshape(BATCH, SEQ, OUT_F)


# revision 9
# speedup vs baseline: 1.0371x; 1.0241x over previous
"""LoRA Linear kernel for Trainium2, 8 NeuronCores.

Computes out = x @ (W + lora_A @ lora_B)^T + bias for
x [4, 2048, 4096], W [4096, 4096], lora_A [4096, 16], lora_B [16, 4096].

Sharding: 2-way over tokens (M = 8192 -> 4096/core) x 4-way over
out_features (4096 -> 1024/core). Operands are bf16 (fp32 PSUM
accumulation); outputs are stored bf16 and upcast on host. The LoRA
delta is folded into W^T on-device with rank-16 matmuls
(Wtot^T = W^T + B^T A^T), added in place over the streamed W^T.

Schedule (v5). The startup critical path is the 8 MiB W^T stream. DMA
efficiency is line-length-bound (2 KiB lines move at ~50 GB/s/queue vs
~300 GB/s for 8 KiB lines), so the host pre-arranges W^T in the exact
SBUF image layout [p, ki, o] and it streams as 8x 1 MiB transfers with
8 KiB lines, striped over the three DMA-capable queues right behind
the first four x tiles. Two lead token-tiles interleave with the fold
at ki pace to cover the stream latency; remaining tiles run os-outer
(first output half adds into a combined per-tile output buffer while
the second accumulates) with a 6-deep PSUM rotation, and each tile
issues one 256 KiB store with 2 KiB lines.
"""

import ml_dtypes

import numpy as np

import concourse.bass as bass
import concourse.bacc as bacc
import concourse.mybir as mybir
import concourse.tile as tile
from concourse.bass_utils import run_bass_kernel_spmd

IN_F = 4096
OUT_F = 4096
RANK = 16
BATCH, SEQ = 4, 2048
M_TOT = BATCH * SEQ          # 8192 tokens
MG, OG = 2, 4                # shard grid: token-groups x outfeature-groups
M_LOC = M_TOT // MG          # 4096 tokens per core
O_LOC = OUT_F // OG          # 1024 out features per core
P = 128
KI = IN_F // P               # 32 contraction tiles
NF = 512                     # matmul moving free dim (one PSUM bank)
OS = O_LOC // NF             # 2 output column passes
MT = M_LOC // P              # 32 token tiles per core
NLEAD = 2
WCH = 4                      # ki tiles per W-stream chunk

F32 = mybir.dt.float32
BF16 = mybir.dt.bfloat16

_cache = {}


def _build():
    nc = bacc.Bacc(None, target_bir_lowering=False)

    # x pre-tiled on host to [MT, P, KI, P]: (mt, i_within, i_tile, m)
    xt = nc.dram_tensor("xt", [MT, P, KI, P], BF16, kind="ExternalInput")
    # W^T pre-arranged on host to the exact wtot SBUF image [p, ki, o]
    wt = nc.dram_tensor("wt", [P, KI, O_LOC], BF16, kind="ExternalInput")
    lb = nc.dram_tensor("lb", [RANK, IN_F], BF16, kind="ExternalInput")
    at = nc.dram_tensor("at", [RANK, O_LOC], BF16, kind="ExternalInput")
    br = nc.dram_tensor("br", [P, O_LOC], BF16, kind="ExternalInput")
    out = nc.dram_tensor("out", [M_LOC, O_LOC], BF16, kind="ExternalOutput")

    with tile.TileContext(nc) as tc:
        with (
            tc.tile_pool(name="const", bufs=1) as const_pool,
            tc.tile_pool(name="xin", bufs=4) as xin_pool,
            tc.tile_pool(name="outs", bufs=3) as out_pool,
            tc.tile_pool(name="psum", bufs=2, space="PSUM") as psum_pool,
            tc.tile_pool(name="psum_mm", bufs=6, space="PSUM") as psum_mm_pool,
        ):
            # resident folded weight, [i_within, i_tile, o] = W^T + B^T A^T
            wtot = const_pool.tile([P, KI, O_LOC], BF16, name="wtot")
            a_sb = const_pool.tile([RANK, O_LOC], BF16, name="a_sb")
            b_sb = const_pool.tile([RANK, IN_F], BF16, name="b_sb")
            bias_sb = const_pool.tile([P, O_LOC], BF16, name="bias_sb")

            x_tiles = {}

            def load_x(mt):
                x_tile = xin_pool.tile([P, KI, P], BF16, name="x_tile", tag="x_tile")
                eng = nc.sync if mt % 2 == 0 else nc.gpsimd
                eng.dma_start(x_tile[:], xt[mt])
                return x_tile

            # ---- DMA issue order matters: per-queue FIFO (only sync /
            # scalar / gpsimd can issue DMAs). Priority: fold constants,
            # x tiles 0-3, then the W^T stream striped over all three
            # queues, then steady-state x.
            nc.gpsimd.dma_start(a_sb[:], at[:])
            nc.gpsimd.dma_start(b_sb[:], lb[:])
            x_tiles[0] = load_x(0)
            x_tiles[1] = load_x(1)
            x_tiles[2] = load_x(2)
            x_tiles[3] = load_x(3)

            # W^T chunks land directly in wtot; the fold adds the rank-16
            # delta in place.
            w_engines = [nc.scalar, nc.sync, nc.gpsimd]
            for c in range(KI // WCH):
                w_engines[c % 3].dma_start(
                    wtot[:, c * WCH : (c + 1) * WCH, :],
                    wt[:, c * WCH : (c + 1) * WCH, :],
                )
            nc.gpsimd.dma_start(bias_sb[:], br[:])

            def new_otile():
                return out_pool.tile([P, O_LOC], BF16, name="o_tile", tag="o_tile")

            def add_bias(os_, psum, o_tile):
                nc.vector.tensor_add(
                    out=o_tile[:, os_ * NF : (os_ + 1) * NF],
                    in0=psum[:],
                    in1=bias_sb[:, os_ * NF : (os_ + 1) * NF],
                )

            def store_tile(mt, o_tile):
                nc.scalar.dma_start(out[mt * P : (mt + 1) * P, :], o_tile[:])

            # ---- m_tiles 0..NLEAD-1, interleaved with the W fold ----
            lead_psums = [
                [
                    psum_mm_pool.tile([P, NF], F32, name=f"psum_{mt}_{os_}", tag="mm")
                    for os_ in range(OS)
                ]
                for mt in range(NLEAD)
            ]
            for ki in range(KI):
                b_ap = b_sb[:, ki * P : (ki + 1) * P]
                for os_ in range(OS):
                    dpsum = psum_pool.tile([P, NF], F32, name="dpsum", tag="dpsum")
                    nc.tensor.matmul(
                        dpsum[:],
                        b_ap,
                        a_sb[:, os_ * NF : (os_ + 1) * NF],
                        start=True,
                        stop=True,
                    )
                    nc.vector.tensor_add(
                        out=wtot[:, ki, os_ * NF : (os_ + 1) * NF],
                        in0=dpsum[:],
                        in1=wtot[:, ki, os_ * NF : (os_ + 1) * NF],
                    )
                for mt in range(NLEAD):
                    for os_ in range(OS):
                        nc.tensor.matmul(
                            lead_psums[mt][os_][:],
                            x_tiles[mt][:, ki, :],
                            wtot[:, ki, os_ * NF : (os_ + 1) * NF],
                            start=(ki == 0),
                            stop=(ki == KI - 1),
                        )
            for mt in range(NLEAD):
                o_tile = new_otile()
                for os_ in range(OS):
                    add_bias(os_, lead_psums[mt][os_], o_tile)
                store_tile(mt, o_tile)

            # ---- m_tiles NLEAD..MT-1, os-outer so os0's bias-add overlaps
            # os1's accumulation; one combined store per tile.
            for mt in range(NLEAD, MT):
                x_tile = x_tiles.get(mt) or load_x(mt)
                o_tile = new_otile()
                for os_ in range(OS):
                    psum = psum_mm_pool.tile([P, NF], F32, name=f"psum{os_}", tag="mm")
                    for ki in range(KI):
                        nc.tensor.matmul(
                            psum[:],
                            x_tile[:, ki, :],
                            wtot[:, ki, os_ * NF : (os_ + 1) * NF],
                            start=(ki == 0),
                            stop=(ki == KI - 1),
                        )
                    add_bias(os_, psum, o_tile)
                store_tile(mt, o_tile)
    nc.finalize()
    return nc


def kernel(x, W, bias, lora_A, lora_B):
    x = np.asarray(x, dtype=np.float32)
    W = np.asarray(W, dtype=np.float32)
    bias = np.asarray(bias, dtype=np.float32)
    lora_A = np.asarray(lora_A, dtype=np.float32)
    lora_B = np.asarray(lora_B, dtype=np.float32)

    if "nc" not in _cache:
        _cache["nc"] = _build()
    nc = _cache["nc"]

    xr = x.reshape(M_TOT, IN_F).astype(ml_dtypes.bfloat16)
    in_maps = []
    for c in range(8):
        mg, og = c % MG, c // MG
        xs = xr[mg * M_LOC : (mg + 1) * M_LOC]
        # [M_LOC, IN_F] -> (mt, m, ki, p) -> (mt, p, ki, m)
        xs = np.ascontiguousarray(xs.reshape(MT, P, KI, P).transpose(0, 3, 2, 1))
        # W^T slice [IN_F, O_LOC] -> (ki, p, o) -> (p, ki, o)
        wslice = W[og * O_LOC : (og + 1) * O_LOC].T.astype(ml_dtypes.bfloat16)
        wslice = np.ascontiguousarray(
            wslice.reshape(KI, P, O_LOC).transpose(1, 0, 2)
        )
        in_maps.append(
            {
                "xt": xs,
                "wt": wslice,
                "lb": np.ascontiguousarray(lora_B.astype(ml_dtypes.bfloat16)),
                "at": np.ascontiguousarray(
                    lora_A[og * O_LOC : (og + 1) * O_LOC].T.astype(ml_dtypes.bfloat16)
                ),
                "br": np.ascontiguousarray(
                    np.broadcast_to(
                        bias[og * O_LOC : (og + 1) * O_LOC].astype(ml_dtypes.bfloat16),
                        (P, O_LOC),
                    )
                ),
            }
        )

    res = run_bass_kernel_spmd(nc, in_maps, core_ids=list(range(8)))

    out = np.empty((M_TOT, OUT_F), dtype=np.float32)
    for c in range(8):
        mg, og = c % MG, c // MG
        out[mg * M_LOC : (mg + 1) * M_LOC, og * O_LOC : (og + 1) * O_LOC] = res.results[
            c
        ]["out"].astype(np.float32)
    return out.reshape(BATCH, SEQ, OUT_F)


# revision 15
# speedup vs baseline: 1.0415x; 1.0042x over previous
"""LoRA Linear kernel for Trainium2, 8 NeuronCores.

Computes out = x @ (W + lora_A @ lora_B)^T + bias for
x [4, 2048, 4096], W [4096, 4096], lora_A [4096, 16], lora_B [16, 4096].

Sharding: 2-way over tokens (M = 8192 -> 4096/core) x 4-way over
out_features (4096 -> 1024/core). Operands are bf16 (fp32 PSUM
accumulation); outputs are stored bf16 and upcast on host. The LoRA
delta is folded into W^T on-device with rank-16 matmuls
(Wtot^T = W^T + B^T A^T), added in place over the streamed W^T.

Schedule (v5). The startup critical path is the 8 MiB W^T stream. DMA
efficiency is line-length-bound (2 KiB lines move at ~50 GB/s/queue vs
~300 GB/s for 8 KiB lines), so the host pre-arranges W^T in the exact
SBUF image layout [p, ki, o] and it streams as 8x 1 MiB transfers with
8 KiB lines, striped over the three DMA-capable queues right behind
the first four x tiles. Two lead token-tiles interleave with the fold
at ki pace to cover the stream latency; remaining tiles run os-outer
(first output half adds into a combined per-tile output buffer while
the second accumulates) with a 6-deep PSUM rotation, and each tile
issues one 256 KiB store with 2 KiB lines.
"""

import ml_dtypes

import numpy as np

import concourse.bass as bass
import concourse.bacc as bacc
import concourse.mybir as mybir
import concourse.tile as tile
from concourse.bass_utils import run_bass_kernel_spmd

IN_F = 4096
OUT_F = 4096
RANK = 16
BATCH, SEQ = 4, 2048
M_TOT = BATCH * SEQ          # 8192 tokens
MG, OG = 2, 4                # shard grid: token-groups x outfeature-groups
M_LOC = M_TOT // MG          # 4096 tokens per core
O_LOC = OUT_F // OG          # 1024 out features per core
P = 128
KI = IN_F // P               # 32 contraction tiles
NF = 512                     # matmul moving free dim (one PSUM bank)
OS = O_LOC // NF             # 2 output column passes
MT = M_LOC // P              # 32 token tiles per core
NLEAD = 3
WCH = 4                      # ki tiles per W-stream chunk

F32 = mybir.dt.float32
BF16 = mybir.dt.bfloat16

_cache = {}


def _build():
    nc = bacc.Bacc(None, target_bir_lowering=False)

    # x pre-tiled on host to [MT, P, KI, P]: (mt, i_within, i_tile, m)
    xt = nc.dram_tensor("xt", [MT, P, KI, P], BF16, kind="ExternalInput")
    # W^T pre-arranged on host to the exact wtot SBUF image [p, ki, o]
    wt = nc.dram_tensor("wt", [P, KI, O_LOC], BF16, kind="ExternalInput")
    lb = nc.dram_tensor("lb", [RANK, IN_F], BF16, kind="ExternalInput")
    at = nc.dram_tensor("at", [RANK, O_LOC], BF16, kind="ExternalInput")
    br = nc.dram_tensor("br", [P, O_LOC], BF16, kind="ExternalInput")
    out = nc.dram_tensor("out", [M_LOC, O_LOC], BF16, kind="ExternalOutput")

    with tile.TileContext(nc) as tc:
        with (
            tc.tile_pool(name="const", bufs=1) as const_pool,
            tc.tile_pool(name="xin", bufs=5) as xin_pool,
            tc.tile_pool(name="outs", bufs=3) as out_pool,
            tc.tile_pool(name="psum", bufs=2, space="PSUM") as psum_pool,
            tc.tile_pool(name="psum_mm", bufs=6, space="PSUM") as psum_mm_pool,
        ):
            # resident folded weight, [i_within, i_tile, o] = W^T + B^T A^T
            wtot = const_pool.tile([P, KI, O_LOC], BF16, name="wtot")
            a_sb = const_pool.tile([RANK, O_LOC], BF16, name="a_sb")
            b_sb = const_pool.tile([RANK, IN_F], BF16, name="b_sb")
            bias_sb = const_pool.tile([P, O_LOC], BF16, name="bias_sb")

            x_tiles = {}

            def load_x(mt):
                x_tile = xin_pool.tile([P, KI, P], BF16, name="x_tile", tag="x_tile")
                eng = nc.sync if mt % 2 == 0 else nc.gpsimd
                eng.dma_start(x_tile[:], xt[mt])
                return x_tile

            # ---- DMA issue order matters: per-queue FIFO (only sync /
            # scalar / gpsimd can issue DMAs). Priority: fold constants,
            # x tiles 0-3, then the W^T stream striped over all three
            # queues, then steady-state x.
            nc.gpsimd.dma_start(a_sb[:], at[:])
            nc.gpsimd.dma_start(b_sb[:], lb[:])
            for mt in range(NLEAD + 1):
                x_tiles[mt] = load_x(mt)

            # W^T chunks land directly in wtot; the fold adds the rank-16
            # delta in place.
            w_engines = [nc.scalar, nc.sync, nc.gpsimd]
            for c in range(KI // WCH):
                w_engines[c % 3].dma_start(
                    wtot[:, c * WCH : (c + 1) * WCH, :],
                    wt[:, c * WCH : (c + 1) * WCH, :],
                )
            nc.gpsimd.dma_start(bias_sb[:], br[:])

            def new_otile():
                return out_pool.tile([P, O_LOC], BF16, name="o_tile", tag="o_tile")

            def add_bias(os_, psum, o_tile):
                nc.vector.tensor_add(
                    out=o_tile[:, os_ * NF : (os_ + 1) * NF],
                    in0=psum[:],
                    in1=bias_sb[:, os_ * NF : (os_ + 1) * NF],
                )

            def store_tile(mt, o_tile):
                nc.scalar.dma_start(out[mt * P : (mt + 1) * P, :], o_tile[:])

            # ---- m_tiles 0..NLEAD-1, interleaved with the W fold ----
            lead_psums = [
                [
                    psum_mm_pool.tile([P, NF], F32, name=f"psum_{mt}_{os_}", tag="mm")
                    for os_ in range(OS)
                ]
                for mt in range(NLEAD)
            ]
            for ki in range(KI):
                b_ap = b_sb[:, ki * P : (ki + 1) * P]
                for os_ in range(OS):
                    dpsum = psum_pool.tile([P, NF], F32, name="dpsum", tag="dpsum")
                    nc.tensor.matmul(
                        dpsum[:],
                        b_ap,
                        a_sb[:, os_ * NF : (os_ + 1) * NF],
                        start=True,
                        stop=True,
                    )
                    nc.vector.tensor_add(
                        out=wtot[:, ki, os_ * NF : (os_ + 1) * NF],
                        in0=dpsum[:],
                        in1=wtot[:, ki, os_ * NF : (os_ + 1) * NF],
                    )
                for mt in range(NLEAD):
                    for os_ in range(OS):
                        nc.tensor.matmul(
                            lead_psums[mt][os_][:],
                            x_tiles[mt][:, ki, :],
                            wtot[:, ki, os_ * NF : (os_ + 1) * NF],
                            start=(ki == 0),
                            stop=(ki == KI - 1),
                        )
            for mt in range(NLEAD):
                o_tile = new_otile()
                for os_ in range(OS):
                    add_bias(os_, lead_psums[mt][os_], o_tile)
                store_tile(mt, o_tile)

            # ---- m_tiles NLEAD..MT-1, os-outer so os0's bias-add overlaps
            # os1's accumulation; one combined store per tile (split per
            # os-half for the final tile to shorten the tail).
            for mt in range(NLEAD, MT):
                x_tile = x_tiles.get(mt) or load_x(mt)
                o_tile = new_otile()
                last = mt == MT - 1
                for os_ in range(OS):
                    psum = psum_mm_pool.tile([P, NF], F32, name=f"psum{os_}", tag="mm")
                    for ki in range(KI):
                        nc.tensor.matmul(
                            psum[:],
                            x_tile[:, ki, :],
                            wtot[:, ki, os_ * NF : (os_ + 1) * NF],
                            start=(ki == 0),
                            stop=(ki == KI - 1),
                        )
                    add_bias(os_, psum, o_tile)
                    if last:
                        nc.scalar.dma_start(
                            out[mt * P : (mt + 1) * P, os_ * NF : (os_ + 1) * NF],
                            o_tile[:, os_ * NF : (os_ + 1) * NF],
                        )
                if not last:
                    store_tile(mt, o_tile)
    nc.finalize()
    return nc


def kernel(x, W, bias, lora_A, lora_B):
    x = np.asarray(x, dtype=np.float32)
    W = np.asarray(W, dtype=np.float32)
    bias = np.asarray(bias, dtype=np.float32)
    lora_A = np.asarray(lora_A, dtype=np.float32)
    lora_B = np.asarray(lora_B, dtype=np.float32)

    if "nc" not in _cache:
        _cache["nc"] = _build()
    nc = _cache["nc"]

    xr = x.reshape(M_TOT, IN_F).astype(ml_dtypes.bfloat16)
    in_maps = []
    for c in range(8):
        mg, og = c % MG, c // MG
        xs = xr[mg * M_LOC : (mg + 1) * M_LOC]
        # [M_LOC, IN_F] -> (mt, m, ki, p) -> (mt, p, ki, m)
        xs = np.ascontiguousarray(xs.reshape(MT, P, KI, P).transpose(0, 3, 2, 1))
        # W^T slice [IN_F, O_LOC] -> (ki, p, o) -> (p, ki, o)
        wslice = W[og * O_LOC : (og + 1) * O_LOC].T.astype(ml_dtypes.bfloat16)
        wslice = np.ascontiguousarray(
            wslice.reshape(KI, P, O_LOC).transpose(1, 0, 2)
        )
        in_maps.append(
            {
                "xt": xs,
                "wt": wslice,
                "lb": np.ascontiguousarray(lora_B.astype(ml_dtypes.bfloat16)),
                "at": np.ascontiguousarray(
                    lora_A[og * O_LOC : (og + 1) * O_LOC].T.astype(ml_dtypes.bfloat16)
                ),
                "br": np.ascontiguousarray(
                    np.broadcast_to(
                        bias[og * O_LOC : (og + 1) * O_LOC].astype(ml_dtypes.bfloat16),
                        (P, O_LOC),
                    )
                ),
            }
        )

    res = run_bass_kernel_spmd(nc, in_maps, core_ids=list(range(8)))

    out = np.empty((M_TOT, OUT_F), dtype=np.float32)
    for c in range(8):
        mg, og = c % MG, c // MG
        out[mg * M_LOC : (mg + 1) * M_LOC, og * O_LOC : (og + 1) * O_LOC] = res.results[
            c
        ]["out"].astype(np.float32)
    return out.reshape(BATCH, SEQ, OUT_F)


# revision 17
# speedup vs baseline: 1.0467x; 1.0051x over previous
"""LoRA Linear kernel for Trainium2, 8 NeuronCores.

Computes out = x @ (W + lora_A @ lora_B)^T + bias for
x [4, 2048, 4096], W [4096, 4096], lora_A [4096, 16], lora_B [16, 4096].

Sharding: 2-way over tokens (M = 8192 -> 4096/core) x 4-way over
out_features (4096 -> 1024/core). Operands are bf16 (fp32 PSUM
accumulation); outputs are stored bf16 and upcast on host. The LoRA
delta is folded into W^T on-device with rank-16 matmuls
(Wtot^T = W^T + B^T A^T), added in place over the streamed W^T.

Schedule (v7). The startup critical path is the 8 MiB W^T stream, and
DMA efficiency is line-length-bound, so the host pre-arranges W^T in
the exact SBUF image layout [p, os, ki, o] (os-major) and it streams as
2x 8 chunks of 4 k-tiles (4 KiB lines) striped over the three
DMA-capable queues. The kernel runs in two half-streams: while the os0
half of W^T lands, three lead token-tiles accumulate their os0 pass
interleaved with the os0 fold at ki pace; ditto for os1. By the time
the leads finish, wtot is fully resident and the remaining 29 tiles run
os-outer at the PE floor with a 6-deep PSUM rotation, one combined
256 KiB store per tile (2 KiB lines). The final tile's stores are
partition-split across two queues to shorten the tail.
"""

import ml_dtypes

import numpy as np

import concourse.bass as bass
import concourse.bacc as bacc
import concourse.mybir as mybir
import concourse.tile as tile
from concourse.bass_utils import run_bass_kernel_spmd

IN_F = 4096
OUT_F = 4096
RANK = 16
BATCH, SEQ = 4, 2048
M_TOT = BATCH * SEQ          # 8192 tokens
MG, OG = 2, 4                # shard grid: token-groups x outfeature-groups
M_LOC = M_TOT // MG          # 4096 tokens per core
O_LOC = OUT_F // OG          # 1024 out features per core
P = 128
KI = IN_F // P               # 32 contraction tiles
NF = 512                     # matmul moving free dim (one PSUM bank)
OS = O_LOC // NF             # 2 output column passes
MT = M_LOC // P              # 32 token tiles per core
NLEAD = 3
WCH = 4                      # ki tiles per W-stream chunk
NCH = KI // WCH              # chunks per os half

F32 = mybir.dt.float32
BF16 = mybir.dt.bfloat16

_cache = {}


def _build():
    nc = bacc.Bacc(None, target_bir_lowering=False)

    # x pre-tiled on host to [MT, P, KI, P]: (mt, i_within, i_tile, m)
    xt = nc.dram_tensor("xt", [MT, P, KI, P], BF16, kind="ExternalInput")
    # W^T pre-arranged on host to the exact wtot SBUF image [p, os, ki, o]
    wt = nc.dram_tensor("wt", [P, OS, KI, NF], BF16, kind="ExternalInput")
    lb = nc.dram_tensor("lb", [RANK, IN_F], BF16, kind="ExternalInput")
    at = nc.dram_tensor("at", [RANK, O_LOC], BF16, kind="ExternalInput")
    br = nc.dram_tensor("br", [P, O_LOC], BF16, kind="ExternalInput")
    out = nc.dram_tensor("out", [M_LOC, O_LOC], BF16, kind="ExternalOutput")

    with tile.TileContext(nc) as tc:
        with (
            tc.tile_pool(name="const", bufs=1) as const_pool,
            tc.tile_pool(name="xin", bufs=5) as xin_pool,
            tc.tile_pool(name="outs", bufs=6) as out_pool,
            tc.tile_pool(name="psum", bufs=2, space="PSUM") as psum_pool,
            tc.tile_pool(name="psum_mm", bufs=6, space="PSUM") as psum_mm_pool,
        ):
            # resident folded weight, [i_within, os, i_tile, o]
            wtot = const_pool.tile([P, OS, KI, NF], BF16, name="wtot")
            a_sb = const_pool.tile([RANK, O_LOC], BF16, name="a_sb")
            b_sb = const_pool.tile([RANK, IN_F], BF16, name="b_sb")
            bias_sb = const_pool.tile([P, O_LOC], BF16, name="bias_sb")

            x_tiles = {}

            def load_x(mt):
                x_tile = xin_pool.tile([P, KI, P], BF16, name="x_tile", tag="x_tile")
                eng = nc.sync if mt % 2 == 0 else nc.gpsimd
                eng.dma_start(x_tile[:], xt[mt])
                return x_tile

            def w_chunk(os_, c, eng):
                eng.dma_start(
                    wtot[:, os_, c * WCH : (c + 1) * WCH, :],
                    wt[:, os_, c * WCH : (c + 1) * WCH, :],
                )

            # ---- DMA issue order matters: per-queue FIFO (only sync /
            # scalar / gpsimd can issue DMAs). a/b land in parallel on two
            # queues, lead x tiles next, then the os0 half of W^T, the
            # next x tiles, the os1 half, then steady-state x.
            nc.gpsimd.dma_start(a_sb[:], at[:])
            nc.scalar.dma_start(b_sb[:], lb[:])
            x_tiles[0] = load_x(0)
            x_tiles[1] = load_x(1)
            x_tiles[2] = load_x(2)
            w_engines = [nc.scalar, nc.sync, nc.gpsimd]
            for c in range(NCH):
                w_chunk(0, c, w_engines[c % 3])
            x_tiles[3] = load_x(3)
            nc.gpsimd.dma_start(bias_sb[:], br[:])
            for c in range(NCH):
                w_chunk(1, c, w_engines[c % 3])
            x_tiles[4] = load_x(4)

            def new_otile():
                return out_pool.tile([P, O_LOC], BF16, name="o_tile", tag="o_tile")

            def add_bias(os_, psum, o_tile):
                nc.vector.tensor_add(
                    out=o_tile[:, os_ * NF : (os_ + 1) * NF],
                    in0=psum[:],
                    in1=bias_sb[:, os_ * NF : (os_ + 1) * NF],
                )

            # ---- lead tiles 0..NLEAD-1: one os half at a time, at ki pace
            # with the fold, so each half-stream's latency is covered.
            lead_otiles = [new_otile() for _ in range(NLEAD)]
            for os_ in range(OS):
                lead_psums = [
                    psum_mm_pool.tile([P, NF], F32, name=f"lpsum_{mt}_{os_}", tag="mm")
                    for mt in range(NLEAD)
                ]
                for ki in range(KI):
                    dpsum = psum_pool.tile([P, NF], F32, name="dpsum", tag="dpsum")
                    nc.tensor.matmul(
                        dpsum[:],
                        b_sb[:, ki * P : (ki + 1) * P],
                        a_sb[:, os_ * NF : (os_ + 1) * NF],
                        start=True,
                        stop=True,
                    )
                    nc.vector.tensor_add(
                        out=wtot[:, os_, ki, :],
                        in0=dpsum[:],
                        in1=wtot[:, os_, ki, :],
                    )
                    for mt in range(NLEAD):
                        nc.tensor.matmul(
                            lead_psums[mt][:],
                            x_tiles[mt][:, ki, :],
                            wtot[:, os_, ki, :],
                            start=(ki == 0),
                            stop=(ki == KI - 1),
                        )
                for mt in range(NLEAD):
                    add_bias(os_, lead_psums[mt], lead_otiles[mt])
            for mt in range(NLEAD):
                nc.scalar.dma_start(out[mt * P : (mt + 1) * P, :], lead_otiles[mt][:])

            # ---- m_tiles NLEAD..MT-1, os-outer so os0's bias-add overlaps
            # os1's accumulation; one combined store per tile. The final
            # tile's stores are partition-split across two queues.
            for mt in range(NLEAD, MT):
                x_tile = x_tiles.get(mt) or load_x(mt)
                o_tile = new_otile()
                for os_ in range(OS):
                    psum = psum_mm_pool.tile([P, NF], F32, name=f"psum{os_}", tag="mm")
                    for ki in range(KI):
                        nc.tensor.matmul(
                            psum[:],
                            x_tile[:, ki, :],
                            wtot[:, os_, ki, :],
                            start=(ki == 0),
                            stop=(ki == KI - 1),
                        )
                    add_bias(os_, psum, o_tile)
                if mt == MT - 1:
                    nc.scalar.dma_start(
                        out[mt * P : mt * P + 64, :], o_tile[0:64, :]
                    )
                    nc.sync.dma_start(
                        out[mt * P + 64 : (mt + 1) * P, :], o_tile[64:128, :]
                    )
                else:
                    nc.scalar.dma_start(out[mt * P : (mt + 1) * P, :], o_tile[:])
    nc.finalize()
    return nc


def kernel(x, W, bias, lora_A, lora_B):
    x = np.asarray(x, dtype=np.float32)
    W = np.asarray(W, dtype=np.float32)
    bias = np.asarray(bias, dtype=np.float32)
    lora_A = np.asarray(lora_A, dtype=np.float32)
    lora_B = np.asarray(lora_B, dtype=np.float32)

    if "nc" not in _cache:
        _cache["nc"] = _build()
    nc = _cache["nc"]

    xr = x.reshape(M_TOT, IN_F).astype(ml_dtypes.bfloat16)
    in_maps = []
    for c in range(8):
        mg, og = c % MG, c // MG
        xs = xr[mg * M_LOC : (mg + 1) * M_LOC]
        # [M_LOC, IN_F] -> (mt, m, ki, p) -> (mt, p, ki, m)
        xs = np.ascontiguousarray(xs.reshape(MT, P, KI, P).transpose(0, 3, 2, 1))
        # W^T slice [IN_F, O_LOC] -> (ki, p, os, o') -> (p, os, ki, o')
        wslice = W[og * O_LOC : (og + 1) * O_LOC].T.astype(ml_dtypes.bfloat16)
        wslice = np.ascontiguousarray(
            wslice.reshape(KI, P, OS, NF).transpose(1, 2, 0, 3)
        )
        in_maps.append(
            {
                "xt": xs,
                "wt": wslice,
                "lb": np.ascontiguousarray(lora_B.astype(ml_dtypes.bfloat16)),
                "at": np.ascontiguousarray(
                    lora_A[og * O_LOC : (og + 1) * O_LOC].T.astype(ml_dtypes.bfloat16)
                ),
                "br": np.ascontiguousarray(
                    np.broadcast_to(
                        bias[og * O_LOC : (og + 1) * O_LOC].astype(ml_dtypes.bfloat16),
                        (P, O_LOC),
                    )
                ),
            }
        )

    res = run_bass_kernel_spmd(nc, in_maps, core_ids=list(range(8)))

    out = np.empty((M_TOT, OUT_F), dtype=np.float32)
    for c in range(8):
        mg, og = c % MG, c // MG
        out[mg * M_LOC : (mg + 1) * M_LOC, og * O_LOC : (og + 1) * O_LOC] = res.results[
            c
        ]["out"].astype(np.float32)
    return out.reshape(BATCH, SEQ, OUT_F)
